# revision 21
# baseline (speedup 1.0000x reference)
"""Biaffine kernel for Trainium2 (8 NeuronCores, Bass/Tile).

out[b,x,y,o] = sum_ij X[b,x,i] w1[i,o,j] Y[b,y,j]
             + (X[b] @ w2[:D])[x,o] + (Y[b] @ w2[D:2D])[y,o] + w2[2D][o]

Sharding: tensor-parallel over o (the w1/w2 out_size axis): core c owns
o in [16c, 16c+16). Each core reads the full (transposed) inputs, its w1/w2
slice, and writes out[b, o_local, x, y]; the host reorders to [b,x,y,o].

Per-core schedule, per (o, batch-pair):
  stage1: M[j, x2] = sum_i W_o[i,j] XT[i, x2]        (PE, fp32r, N=512)
  fold:   M += w2b[j,o]      (DVE tensor_scalar_add on PSUM->SBUF copy)
  stage2: out[x, y] = sum_j M[j, x] YT[j, y]         (PE, fp32r, N=256)
  fold:   out += c1b[x, o]   (ACT Identity+bias on PSUM->SBUF copy)
where c1b = X[b] @ w2a + bias is computed once per (b, x-tile) upfront.
"""

import numpy as np

B, L, D, O = 8, 256, 512, 128
NCORES = 8
OS = O // NCORES     # 16 o-channels per core
IC = D // 128        # 4 contraction chunks of 128
NBP = B // 2         # 4 batch pairs (stage1 moving dim = 2*L = 512)

_CACHE = {}


def _build(mmdt_name: str, n_reps: int = 1, variant: str = "full"):
    """Build + compile the per-core Bass program (same program on all cores).

    n_reps > 1 repeats the main loop inside the NEFF (timing amplification
    for benchmarking only; results are identical since it rewrites the same
    outputs).
    """
    import concourse.tile as tile
    from concourse import bacc, mybir

    key = (mmdt_name, n_reps, variant)
    if key in _CACHE:
        return _CACHE[key]

    F32 = mybir.dt.float32
    MMDT = getattr(mybir.dt, mmdt_name)

    nc = bacc.Bacc("TRN2", target_bir_lowering=False, debug=False,
                   num_devices=NCORES)

    x1t_d = nc.dram_tensor("x1t", [B, D, L], MMDT, kind="ExternalInput")
    x2t_d = nc.dram_tensor("x2t", [B, D, L], MMDT, kind="ExternalInput")
    w1s_d = nc.dram_tensor("w1s", [OS, IC, 128, D], MMDT, kind="ExternalInput")
    w2a_d = nc.dram_tensor("w2a", [128, IC, OS], MMDT, kind="ExternalInput")
    w2b_d = nc.dram_tensor("w2b", [128, IC, OS], F32, kind="ExternalInput")
    bias_d = nc.dram_tensor("bias", [1, OS], MMDT, kind="ExternalInput")
    ones_d = nc.dram_tensor("ones", [1, 128], MMDT, kind="ExternalInput")
    out_d = nc.dram_tensor("out", [B, OS, L, L], F32, kind="ExternalOutput")


    if variant == "stub":
        # minimal program with identical I/O signature (absolute-timing baseline)
        with tile.TileContext(nc) as tc:
            with tc.tile_pool(name="sb", bufs=1) as sb:
                t = sb.tile([128, IC, OS], F32, tag="t")
                o_sb = sb.tile([128, L], F32, tag="o")
                nc.sync.dma_start(out=t, in_=w2b_d.ap())
                nc.vector.memset(o_sb, 0.0)
                nc.vector.tensor_scalar_add(o_sb[:, 0:OS * IC],
                                            o_sb[:, 0:OS * IC], t[:, 0, 0:1])
                nc.sync.dma_start(out=out_d.ap()[0, 0, 0:128, :], in_=o_sb)
        nc.compile()
        _CACHE[key] = nc
        return nc

    ps2_bufs = 6 if variant in ("v2", "o8", "v4", "v5", "v6") else 4
    with tile.TileContext(nc) as tc:
        with tc.tile_pool(name="small", bufs=1) as small, \
             tc.tile_pool(name="xy", bufs=1) as xy, \
             tc.tile_pool(name="wp", bufs=4) as wp, \
             tc.tile_pool(name="mp", bufs=3) as mp, \
             tc.tile_pool(name="op", bufs=8) as op, \
             tc.tile_pool(name="ps1", bufs=2, space="PSUM") as ps1:

            # --- small persistent tiles ---
            w2a_sb = small.tile([128, IC, OS], MMDT, tag="w2a")
            w2b_sb = small.tile([128, IC, OS], F32, tag="w2b")
            bias_sb = small.tile([1, OS], MMDT, tag="bias")
            ones_sb = small.tile([1, 128], MMDT, tag="ones")
            c1b_sb = small.tile([128, B * 2 * OS], F32, tag="c1b")
            nc.sync.dma_start(out=w2a_sb, in_=w2a_d.ap())
            nc.sync.dma_start(out=w2b_sb, in_=w2b_d.ap())
            nc.sync.dma_start(out=bias_sb, in_=bias_d.ap())
            nc.sync.dma_start(out=ones_sb, in_=ones_d.ap())

            # --- prefetch first W tiles so the o=0 stage1 isn't gated on the
            # (larger) input loads finishing first on the same DMA queue ---
            w_cache = {}
            # v5: W loads ride the Activation HWDGE ring so they never queue
            # behind the (in-order) sync ring's input loads + output stores
            w_eng = nc.scalar if variant == "v5" else nc.sync
            if variant in ("v2", "o8", "v3", "v4", "v5", "v6"):
                for o in range(2):
                    w_t = []
                    for ic in range(IC):
                        w = wp.tile([128, D], MMDT, tag=f"w{ic}")
                        w_eng.dma_start(out=w, in_=w1s_d.ap()[o, ic, :, :])
                        w_t.append(w)
                    w_cache[o] = w_t

            # --- transposed inputs, all batches resident: [i%128, ic, b_in, x] ---
            xts, yts = [], []
            for bp in range(NBP):
                xt = xy.tile([128, IC, 2, L], MMDT, tag=f"xt{bp}")
                yt = xy.tile([128, IC, 2, L], MMDT, tag=f"yt{bp}")
                for b_in in range(2):
                    b = 2 * bp + b_in
                    for ic in range(IC):
                        in_eng = nc.scalar if (variant == "v5" and ic % 2) \
                            else nc.sync
                        in_eng.dma_start(
                            out=xt[:, ic, b_in, :],
                            in_=x1t_d.ap()[b, ic * 128:(ic + 1) * 128, :])
                        in_eng.dma_start(
                            out=yt[:, ic, b_in, :],
                            in_=x2t_d.ap()[b, ic * 128:(ic + 1) * 128, :])
                xts.append(xt)
                yts.append(yt)

            # --- c1b[x, (b,xt,o)] = X[b] @ w2a + bias ---
            psc_pool = [None]

            def emit_c1b(b):
                psc = psc_pool[0]
                bp, b_in = divmod(b, 2)
                for xt_i in range(2):
                    # v4 borrows stage2's psum slots (same tag) so no extra
                    # PSUM banks are reserved for this startup-only work
                    pc = psc.tile([128, OS], F32,
                                  tag="p2" if variant == "v4" else "pc")
                    for ic in range(IC):
                        nc.tensor.matmul(
                            pc,
                            xts[bp][:, ic, b_in, xt_i * 128:(xt_i + 1) * 128],
                            w2a_sb[:, ic, :],
                            start=(ic == 0), stop=False)
                    nc.tensor.matmul(
                        pc, ones_sb[0:1, :], bias_sb[0:1, :],
                        start=False, stop=True)
                    nc.vector.tensor_copy(
                        c1b_sb[:, (b * 2 + xt_i) * OS:(b * 2 + xt_i + 1) * OS],
                        pc)

            if variant not in ("v3", "v4"):
                with tc.tile_pool(name="psc", bufs=2, space="PSUM") as psc:
                    psc_pool[0] = psc
                    for b in range(B):
                        emit_c1b(b)

            if variant == "v3":
                # c1b psum shares the pool budget with ps2 (2 + 4 + 2 = 8 banks)
                psc_ctx = tc.tile_pool(name="psc", bufs=2, space="PSUM")
                psc_pool[0] = psc_ctx.__enter__()
                ps2_bufs = 4
            ps2_ctx = tc.tile_pool(name="ps2", bufs=ps2_bufs, space="PSUM")
            ps2 = ps2_ctx.__enter__()
            if variant == "v4":
                psc_pool[0] = ps2

            # --- main loop: software-pipelined over (o, bp) ---
            def stage2(o, bp, m2):
                for b_in in range(2):
                    b = 2 * bp + b_in
                    for xt_i in range(2):
                        p2 = ps2.tile([128, L], F32, tag="p2")
                        for jc in range(IC):
                            nc.tensor.matmul(
                                p2,
                                m2[:, jc, b_in, xt_i * 128:(xt_i + 1) * 128],
                                yts[bp][:, jc, b_in, :],
                                start=(jc == 0), stop=(jc == IC - 1))
                        o_sb = op.tile([128, L], F32, tag="osb")
                        c1col = c1b_sb[:, (b * 2 + xt_i) * OS + o:
                                       (b * 2 + xt_i) * OS + o + 1]
                        if variant == "dvecopy":
                            nc.vector.tensor_scalar_add(o_sb, p2, c1col)
                        elif variant == "v6" and xt_i == 1:
                            # balance stage2 psum drains across DVE and ACT so
                            # neither engine gates ps2 slot reuse
                            nc.vector.tensor_scalar_add(o_sb, p2, c1col)
                        else:
                            nc.scalar.add(o_sb, p2, c1col)
                        if variant != "nodma" or (o == OS - 1 and bp == NBP - 1):
                            nc.sync.dma_start(
                                out=out_d.ap()[b, o, xt_i * 128:(xt_i + 1) * 128, :],
                                in_=o_sb)

            def stage1(o, bp, w_t):
                m2 = mp.tile([128, IC, 2, L], MMDT, tag="m2")
                for jt in range(IC):
                    p1 = ps1.tile([128, 2 * L], F32, tag="p1")
                    for ic in range(IC):
                        nc.tensor.matmul(
                            p1,
                            w_t[ic][:, jt * 128:(jt + 1) * 128],
                            xts[bp][:, ic, :, :],
                            start=(ic == 0), stop=(ic == IC - 1))
                    nc.vector.tensor_scalar_add(
                        m2[:, jt, :, :], p1, w2b_sb[:, jt, o:o + 1])
                return m2

            os_eff = OS // 2 if variant == "o8" else OS

            def emit_main():
                # software-pipelined: stage2 for (o,bp) runs one step behind
                # stage1 so the PE never waits on the DVE M-copies.
                prev = None
                for o in range(os_eff):
                    if o in w_cache:
                        w_t = w_cache.pop(o)
                    else:
                        w_t = []
                        for ic in range(IC):
                            w = wp.tile([128, D], MMDT, tag=f"w{ic}")
                            w_eng.dma_start(out=w, in_=w1s_d.ap()[o, ic, :, :])
                            w_t.append(w)
                    for bp in range(NBP):
                        m2 = stage1(o, bp, w_t)
                        if variant in ("v3", "v4") and o == 0:
                            emit_c1b(2 * bp)
                            emit_c1b(2 * bp + 1)
                        if prev is not None:
                            stage2(*prev)
                        prev = (o, bp, m2)
                stage2(*prev)

            if n_reps == 1:
                emit_main()
            else:
                with tc.For_i(0, n_reps, 1):
                    emit_main()
            ps2_ctx.__exit__(None, None, None)
            if variant == "v3":
                psc_ctx.__exit__(None, None, None)

    nc.compile()
    _CACHE[key] = nc
    return nc


def _build2(n_reps: int = 1, variant: str = "b16", timing: bool = False,
            os_eff: int = OS):
    """bf16 rewrite: same two-stage schedule as v2, with
      - all matmul operands bf16 (host converts; PE still 1 cyc/row but
        operand/weight-load SBUF bandwidth halves — the fp32r stage2
        N=256 case is right at the fp32r ldweights-hiding margin),
      - bf16 output stores (host upconverts; halves the dominant DMA
        stream), fp32 accumulation throughout (PSUM),
      - coalesced input/weight DMAs (one descriptor-chain per tile),
      - w1 tiles ride the ACT HWDGE ring, prefetch depth 3,
      - c1b warmup interleaved into the o=0 stage1 loop (v4-style).

    timing=True builds a NEFF with Internal (scratch) DRAM for the big
    tensors and a tiny external output, so paired wall-clock timing
    doesn't ship ~450MB through the axon tunnel per call. The main-loop
    instruction stream is identical to timing=False.
    """
    import concourse.tile as tile
    from concourse import bacc, mybir

    key = ("b16", n_reps, variant, timing, os_eff)
    if key in _CACHE:
        return _CACHE[key]

    F32 = mybir.dt.float32
    BF16 = mybir.dt.bfloat16

    nc = bacc.Bacc("TRN2", target_bir_lowering=False, debug=False,
                   num_devices=NCORES)

    big = dict(kind="Internal") if timing else dict(kind="ExternalInput")
    big_out = dict(kind="Internal") if timing else dict(kind="ExternalOutput")

    # host-marshaled layouts: partition dim (128) first, fully contiguous
    # per-partition payload per DMA
    x1t_d = nc.dram_tensor("x1t", [128, NBP, IC, 2, L], BF16, **big)
    x2t_d = nc.dram_tensor("x2t", [128, NBP, IC, 2, L], BF16, **big)
    w1s_d = nc.dram_tensor("w1s", [128, OS, IC, D], BF16, **big)
    w2a_d = nc.dram_tensor("w2a", [128, IC, OS], BF16, kind="ExternalInput")
    w2b_d = nc.dram_tensor("w2b", [128, IC, OS], F32, kind="ExternalInput")
    bias_d = nc.dram_tensor("bias", [1, OS], BF16, kind="ExternalInput")
    ones_d = nc.dram_tensor("ones", [1, 128], BF16, kind="ExternalInput")
    out_d = nc.dram_tensor("out", [B, OS, L, L], BF16, **big_out)
    if timing:
        tout_d = nc.dram_tensor("tout", [1, 128], BF16, kind="ExternalOutput")

    with tile.TileContext(nc) as tc:
        with tc.tile_pool(name="small", bufs=1) as small, \
             tc.tile_pool(name="xy", bufs=1) as xy, \
             tc.tile_pool(name="wp", bufs=4) as wp, \
             tc.tile_pool(name="mp", bufs=4) as mp, \
             tc.tile_pool(name="op", bufs=8) as op, \
             tc.tile_pool(name="ps1", bufs=3, space="PSUM") as ps1, \
             tc.tile_pool(name="ps2", bufs=5, space="PSUM") as ps2:

            # --- bp0 inputs lead the sync ring: stage1(o=0,bp=0) is gated
            # only on xt0 + w(o=0), everything else loads behind them ---
            xts, yts = [], []
            for bp in range(NBP):
                xts.append(xy.tile([128, IC, 2, L], BF16, tag=f"xt{bp}",
                                   name=f"xt{bp}"))
                yts.append(xy.tile([128, IC, 2, L], BF16, tag=f"yt{bp}",
                                   name=f"yt{bp}"))
            for ic in range(IC):
                nc.sync.dma_start(out=xts[0][:, ic], in_=x1t_d.ap()[:, 0, ic])
            nc.sync.dma_start(out=yts[0], in_=x2t_d.ap()[:, 0])

            # --- w1 tiles ride the ACT ring (never queue behind inputs);
            # the first tile is split across the ACT+Pool rings so its
            # transfer overlaps the xt0 load ---
            def load_w(o, split=False):
                w = wp.tile([128, IC, D], BF16, tag="w", name="w")
                if split:
                    nc.scalar.dma_start(out=w[:, 0:2, :],
                                        in_=w1s_d.ap()[:, o, 0:2, :])
                    nc.gpsimd.dma_start(out=w[:, 2:4, :],
                                        in_=w1s_d.ap()[:, o, 2:4, :])
                else:
                    nc.scalar.dma_start(out=w, in_=w1s_d.ap()[:, o])
                return w
            w_fifo = [load_w(o, split=(o == 0))
                      for o in range(min(3, os_eff))]

            # --- small persistent tiles ride the Pool ring (own DGE) ---
            w2a_sb = small.tile([128, IC, OS], BF16, tag="w2a")
            w2b_sb = small.tile([128, IC, OS], F32, tag="w2b")
            bias_sb = small.tile([1, OS], BF16, tag="bias")
            ones_sb = small.tile([1, 128], BF16, tag="ones")
            c1b_sb = small.tile([128, B * 2 * OS], F32, tag="c1b")
            nc.gpsimd.dma_start(out=w2a_sb, in_=w2a_d.ap())
            nc.gpsimd.dma_start(out=w2b_sb, in_=w2b_d.ap())
            nc.gpsimd.dma_start(out=bias_sb, in_=bias_d.ap())
            nc.gpsimd.dma_start(out=ones_sb, in_=ones_d.ap())

            # --- remaining inputs ---
            for bp in range(1, NBP):
                nc.sync.dma_start(out=xts[bp], in_=x1t_d.ap()[:, bp])
                nc.sync.dma_start(out=yts[bp], in_=x2t_d.ap()[:, bp])

            def emit_c1b(b):
                bp, b_in = divmod(b, 2)
                for xt_i in range(2):
                    pc = ps2.tile([128, L], F32, tag="p2")
                    for ic in range(IC):
                        nc.tensor.matmul(
                            pc[:, 0:OS],
                            xts[bp][:, ic, b_in, xt_i * 128:(xt_i + 1) * 128],
                            w2a_sb[:, ic, :],
                            start=(ic == 0), stop=False)
                    nc.tensor.matmul(
                        pc[:, 0:OS], ones_sb[0:1, :], bias_sb[0:1, :],
                        start=False, stop=True)
                    nc.vector.tensor_copy(
                        c1b_sb[:, (b * 2 + xt_i) * OS:(b * 2 + xt_i + 1) * OS],
                        pc[:, 0:OS])

            def stage1(o, bp, w_t):
                m2 = mp.tile([128, IC, 2, L], BF16, tag="m2")
                for jt in range(IC):
                    p1 = ps1.tile([128, 2 * L], F32, tag="p1")
                    for ic in range(IC):
                        nc.tensor.matmul(
                            p1,
                            w_t[:, ic, jt * 128:(jt + 1) * 128],
                            xts[bp][:, ic, :, :],
                            start=(ic == 0), stop=(ic == IC - 1))
                    nc.vector.tensor_scalar_add(
                        m2[:, jt, :, :], p1, w2b_sb[:, jt, o:o + 1])
                return m2

            def stage2(o, bp, m2):
                for b_in in range(2):
                    b = 2 * bp + b_in
                    for xt_i in range(2):
                        p2 = ps2.tile([128, L], F32, tag="p2")
                        for jc in range(IC):
                            nc.tensor.matmul(
                                p2,
                                m2[:, jc, b_in, xt_i * 128:(xt_i + 1) * 128],
                                yts[bp][:, jc, b_in, :],
                                start=(jc == 0), stop=(jc == IC - 1))
                        o_sb = op.tile([128, L], BF16, tag="osb")
                        c1col = c1b_sb[:, (b * 2 + xt_i) * OS + o:
                                       (b * 2 + xt_i) * OS + o + 1]
                        nc.scalar.add(o_sb, p2, c1col)
                        nc.sync.dma_start(
                            out=out_d.ap()[b, o, xt_i * 128:(xt_i + 1) * 128, :],
                            in_=o_sb)
                return o_sb

            last_osb = [None]

            def emit_main():
                prev = None
                for o in range(os_eff):
                    w_t = w_fifo[o % len(w_fifo)]
                    if o + 3 < os_eff:
                        w_fifo[(o + 3) % len(w_fifo)] = load_w(o + 3)
                    for bp in range(NBP):
                        m2 = stage1(o, bp, w_t)
                        if o == 0:
                            emit_c1b(2 * bp)
                            emit_c1b(2 * bp + 1)
                        if prev is not None:
                            stage2(*prev)
                        prev = (o, bp, m2)
                last_osb[0] = stage2(*prev)

            if n_reps == 1:
                emit_main()
            else:
                with tc.For_i(0, n_reps, 1):
                    emit_main()

            if timing:
                nc.sync.dma_start(out=tout_d.ap(),
                                  in_=last_osb[0][0:1, :][:, 0:128])

    nc.compile()
    _CACHE[key] = nc
    return nc


def _build3(n_reps: int = 1, timing: bool = False, os_eff: int = OS,
            warm: int = 24):
    """b16s: bf16 two-stage schedule with a swapped stage2.

    stage1 (unchanged): p1[j, (b_in,x)] = sum_i W_o[i,j] X[i, (b_in,x)],
      DVE drains +w2b[j,o] into m2[j, jc, b_in, oi, x] (bf16), two o's
      (an "o-pair") share one m2 tile.
    stage2 (swapped): stationary = Y chunk [j, y128], moving = m2 slice
      [j, (oi,x)=512] -> p2[y128, (oi,x)] accumulated over 4 j-chunks.
      vs the b16 layout this halves stage2 matmul/ldweights/drain/store
      counts and doubles the moving dim (better weight-load hiding).
    The x-affine term (X@w2a + bias) no longer folds as a per-partition
    scalar (partitions are now y), so the HOST adds it during unshard;
    the y-affine (Y@w2b) still rides stage1's +w2b fold.
    A short burst of dummy matmuls at t=0 ramps the PE p-state out of
    its cold clock while the first DMAs land.
    """
    import concourse.tile as tile
    from concourse import bacc, mybir

    key = ("b16s", n_reps, timing, os_eff, warm)
    if key in _CACHE:
        return _CACHE[key]

    F32 = mybir.dt.float32
    BF16 = mybir.dt.bfloat16
    OP2 = os_eff // 2

    nc = bacc.Bacc("TRN2", target_bir_lowering=False, debug=False,
                   num_devices=NCORES)

    big = dict(kind="Internal") if timing else dict(kind="ExternalInput")
    big_out = dict(kind="Internal") if timing else dict(kind="ExternalOutput")

    x1t_d = nc.dram_tensor("x1t", [128, NBP, IC, 2, L], BF16, **big)
    x2t_d = nc.dram_tensor("x2t", [128, NBP, IC, 2, L], BF16, **big)
    w1s_d = nc.dram_tensor("w1s", [128, OS, IC, D], BF16, **big)
    w2b_d = nc.dram_tensor("w2b", [128, IC, OS], F32, kind="ExternalInput")
    out_d = nc.dram_tensor("out", [B, OS // 2, 2, 128, 2 * L], BF16,
                           **big_out)
    if timing:
        tout_d = nc.dram_tensor("tout", [1, 128], BF16, kind="ExternalOutput")

    with tile.TileContext(nc) as tc:
        with tc.tile_pool(name="small", bufs=1) as small, \
             tc.tile_pool(name="xy", bufs=1) as xy, \
             tc.tile_pool(name="wp", bufs=4) as wp, \
             tc.tile_pool(name="mp", bufs=3) as mp, \
             tc.tile_pool(name="op", bufs=8) as op, \
             tc.tile_pool(name="ps1", bufs=3, space="PSUM") as ps1, \
             tc.tile_pool(name="ps2", bufs=4, space="PSUM") as ps2, \
             tc.tile_pool(name="psw", bufs=1, space="PSUM") as psw:

            # --- PE p-state warmup: zero matmuls, gated only on one memset ---
            dum = small.tile([128, 128], BF16, tag="dum", name="dum")
            nc.vector.memset(dum, 0.0)
            for _ in range(warm):
                pd = psw.tile([128, 2 * L], F32, tag="pw", name="pw")
                nc.tensor.matmul(pd[:, 0:128], dum, dum,
                                 start=True, stop=True)

            # --- bp0 inputs lead the sync ring, per-ic for fine gating ---
            xts, yts = [], []
            for bp in range(NBP):
                xts.append(xy.tile([128, IC, 2, L], BF16, tag=f"xt{bp}",
                                   name=f"xt{bp}"))
                yts.append(xy.tile([128, IC, 2, L], BF16, tag=f"yt{bp}",
                                   name=f"yt{bp}"))
            for ic in range(IC):
                nc.sync.dma_start(out=xts[0][:, ic], in_=x1t_d.ap()[:, 0, ic])

            # --- w1 tiles on the ACT ring; first split ACT+Pool ---
            def load_w(o, split=False):
                w = wp.tile([128, IC, D], BF16, tag="w", name="w")
                if split:
                    nc.scalar.dma_start(out=w[:, 0:2, :],
                                        in_=w1s_d.ap()[:, o, 0:2, :])
                    nc.gpsimd.dma_start(out=w[:, 2:4, :],
                                        in_=w1s_d.ap()[:, o, 2:4, :])
                else:
                    nc.scalar.dma_start(out=w, in_=w1s_d.ap()[:, o])
                return w
            # n_reps==1: prefetch 3 of 4 slots before the main loop.
            # n_reps>1 (timing builds): NO pre-loop w tiles — tiles read
            # inside a For_i body but loaded outside are pinned for the
            # loop's lifetime, so in-body loads couldn't recycle their
            # slots; instead every load (16/rep, matching the real
            # kernel's DMA) is emitted inside the body.
            w_tiles = {}
            if n_reps == 1:
                for o in range(min(3, os_eff)):
                    w_tiles[o] = load_w(o, split=(o == 0))

            w2b_sb = small.tile([128, IC, OS], F32, tag="w2b")
            nc.gpsimd.dma_start(out=w2b_sb, in_=w2b_d.ap())

            # xt1 ahead of yt0: stage1(0,bp1) consumes it well before
            # stage2(0,bp0) needs yt0
            nc.sync.dma_start(out=xts[1], in_=x1t_d.ap()[:, 1])
            nc.sync.dma_start(out=yts[0], in_=x2t_d.ap()[:, 0])
            nc.sync.dma_start(out=yts[1], in_=x2t_d.ap()[:, 1])
            for bp in range(2, NBP):
                nc.sync.dma_start(out=xts[bp], in_=x1t_d.ap()[:, bp])
                nc.sync.dma_start(out=yts[bp], in_=x2t_d.ap()[:, bp])

            def stage1(o, bp, w_t, m2, oi):
                for jt in range(IC):
                    p1 = ps1.tile([128, 2 * L], F32, tag="p1", name="p1")
                    for ic in range(IC):
                        nc.tensor.matmul(
                            p1,
                            w_t[:, ic, jt * 128:(jt + 1) * 128],
                            xts[bp][:, ic, :, :],
                            start=(ic == 0), stop=(ic == IC - 1))
                    nc.vector.tensor_scalar_add(
                        m2[:, jt, :, oi, :], p1, w2b_sb[:, jt, o:o + 1])

            def stage2(opair, bp, m2):
                for b_in in range(2):
                    b = 2 * bp + b_in
                    for yc in range(2):
                        p2 = ps2.tile([128, 2 * L], F32, tag="p2", name="p2")
                        for jc in range(IC):
                            nc.tensor.matmul(
                                p2,
                                yts[bp][:, jc, b_in, yc * 128:(yc + 1) * 128],
                                m2[:, jc, b_in, :, :],
                                start=(jc == 0), stop=(jc == IC - 1))
                        o_sb = op.tile([128, 2 * L], BF16, tag="osb",
                                       name="osb")
                        nc.scalar.copy(o_sb, p2)
                        nc.sync.dma_start(out=out_d.ap()[b, opair, yc],
                                          in_=o_sb)
                return o_sb

            last_osb = [None]

            def emit_main(in_loop):
                prev = None
                if in_loop:
                    for o in range(min(4, os_eff)):
                        w_tiles[o] = load_w(o)
                for opair in range(OP2):
                    o0, o1 = 2 * opair, 2 * opair + 1
                    wa, wb = w_tiles[o0], w_tiles[o1]
                    for bp in range(NBP):
                        m2 = mp.tile([128, IC, 2, 2, L], BF16, tag="m2",
                                     name="m2")
                        stage1(o0, bp, wa, m2, 0)
                        stage1(o1, bp, wb, m2, 1)
                        if prev is not None:
                            stage2(*prev)
                        prev = (opair, bp, m2)
                    # issue loads only after every stage1 read of the slot
                    # they recycle (keeps each wp slot read-before-write
                    # within the For_i body, which the tile scheduler
                    # requires)
                    base = 2 * opair + (4 if in_loop else 3)
                    for oo in (base, base + 1):
                        if oo < os_eff and oo not in w_tiles:
                            w_tiles[oo] = load_w(oo)
                last_osb[0] = stage2(*prev)

            if n_reps == 1:
                emit_main(False)
            else:
                with tc.For_i(0, n_reps, 1):
                    emit_main(True)

            if timing:
                nc.sync.dma_start(out=tout_d.ap(),
                                  in_=last_osb[0][0:1, 0:128])

    nc.compile()
    _CACHE[key] = nc
    return nc


def make_in_maps3(input1, input2, w1, w2, timing: bool = False):
    """Host-side marshaling for _build3 (b16s)."""
    import ml_dtypes

    bf16 = ml_dtypes.bfloat16
    input1 = np.asarray(input1, dtype=np.float32)
    input2 = np.asarray(input2, dtype=np.float32)
    w1 = np.asarray(w1, dtype=np.float32)
    w2 = np.asarray(w2, dtype=np.float32)

    def xmarsh(x):
        v = x.reshape(NBP, 2, L, IC, 128)          # [bp, b_in, l, ic, p]
        return np.ascontiguousarray(
            v.transpose(4, 0, 3, 1, 2)).astype(bf16)  # [p, bp, ic, b_in, l]

    x1t = xmarsh(input1)
    x2t = xmarsh(input2)

    in_maps = []
    for c in range(NCORES):
        sl = slice(c * OS, (c + 1) * OS)
        w2b = np.ascontiguousarray(
            w2[D:2 * D, sl].reshape(IC, 128, OS).transpose(1, 0, 2))
        m = {"w2b": w2b}
        if not timing:
            w1s = np.ascontiguousarray(
                w1[:, sl, :].reshape(IC, 128, OS, D).transpose(1, 2, 0, 3)
            ).astype(bf16)
            m.update({"x1t": x1t, "x2t": x2t, "w1s": w1s})
        in_maps.append(m)
    return in_maps


def make_in_maps2(input1, input2, w1, w2, timing: bool = False):
    """Host-side marshaling for _build2 (bf16, partition-major layouts)."""
    import ml_dtypes

    bf16 = ml_dtypes.bfloat16
    input1 = np.asarray(input1, dtype=np.float32)
    input2 = np.asarray(input2, dtype=np.float32)
    w1 = np.asarray(w1, dtype=np.float32)
    w2 = np.asarray(w2, dtype=np.float32)

    # x1t[p, bp, ic, b_in, l] = input1[2bp+b_in, l, ic*128+p]
    def xmarsh(x):
        v = x.reshape(NBP, 2, L, IC, 128)          # [bp, b_in, l, ic, p]
        return np.ascontiguousarray(
            v.transpose(4, 0, 3, 1, 2)).astype(bf16)  # [p, bp, ic, b_in, l]

    x1t = xmarsh(input1)
    x2t = xmarsh(input2)
    ones = np.ones((1, 128), dtype=bf16)

    in_maps = []
    for c in range(NCORES):
        sl = slice(c * OS, (c + 1) * OS)
        w2a = np.ascontiguousarray(
            w2[:D, sl].reshape(IC, 128, OS).transpose(1, 0, 2)).astype(bf16)
        w2b = np.ascontiguousarray(
            w2[D:2 * D, sl].reshape(IC, 128, OS).transpose(1, 0, 2))
        bias = np.ascontiguousarray(w2[2 * D:2 * D + 1, sl]).astype(bf16)
        m = {"w2a": w2a, "w2b": w2b, "bias": bias, "ones": ones}
        if not timing:
            # w1s[p, o, ic, j] = w1[ic*128+p, o_global, j]
            w1s = np.ascontiguousarray(
                w1[:, sl, :].reshape(IC, 128, OS, D).transpose(1, 2, 0, 3)
            ).astype(bf16)
            m.update({"x1t": x1t, "x2t": x2t, "w1s": w1s})
        in_maps.append(m)
    return in_maps


def make_in_maps(input1, input2, w1, w2):
    """Host-side data marshaling (sharding + layout)."""
    input1 = np.asarray(input1, dtype=np.float32)
    input2 = np.asarray(input2, dtype=np.float32)
    w1 = np.asarray(w1, dtype=np.float32)
    w2 = np.asarray(w2, dtype=np.float32)

    x1t = np.ascontiguousarray(input1.transpose(0, 2, 1))      # [B, D, L]
    x2t = np.ascontiguousarray(input2.transpose(0, 2, 1))      # [B, D, L]
    ones = np.ones((1, 128), dtype=np.float32)

    in_maps = []
    for c in range(NCORES):
        sl = slice(c * OS, (c + 1) * OS)
        w1s = np.ascontiguousarray(
            w1[:, sl, :].transpose(1, 0, 2)).reshape(OS, IC, 128, D)
        # SBUF layout is [i_in_chunk(128 partitions), chunk, o]
        w2a = np.ascontiguousarray(
            w2[:D, sl].reshape(IC, 128, OS).transpose(1, 0, 2))
        w2b = np.ascontiguousarray(
            w2[D:2 * D, sl].reshape(IC, 128, OS).transpose(1, 0, 2))
        bias = np.ascontiguousarray(w2[2 * D:2 * D + 1, sl])
        in_maps.append({"x1t": x1t, "x2t": x2t, "w1s": w1s,
                        "w2a": w2a, "w2b": w2b, "bias": bias, "ones": ones})
    return in_maps


def kernel(input1, input2, w1, w2):
    from concourse.bass_utils import run_bass_kernel_spmd

    in_maps = make_in_maps3(input1, input2, w1, w2)
    nc = _build3(1)
    res = run_bass_kernel_spmd(nc, in_maps, core_ids=list(range(NCORES)))

    out = np.empty((B, L, L, O), dtype=np.float32)
    for c in range(NCORES):
        # per-core result [B, op(8), yc(2), y'(128), (oi,x)=512] bf16
        a = np.asarray(res.results[c]["out"]).astype(np.float32)
        a = a.reshape(B, OS // 2, 2, 128, 2, L)
        out[:, :, :, c * OS:(c + 1) * OS] = \
            a.transpose(0, 5, 2, 3, 1, 4).reshape(B, L, L, OS)

    # x-affine term (X @ w2a + bias), folded on host: with stage2's output
    # partitions = y it is no longer a per-partition scalar on-device.
    w2 = np.asarray(w2, dtype=np.float32)
    c1 = (np.asarray(input1, np.float32).reshape(B * L, D) @ w2[:D]
          + w2[2 * D])
    out += c1.reshape(B, L, 1, O)
    return out



# revision 24
# speedup vs baseline: 1.1489x; 1.1489x over previous
"""Biaffine kernel for Trainium2 (8 NeuronCores, Bass/Tile).

out[b,x,y,o] = sum_ij X[b,x,i] w1[i,o,j] Y[b,y,j]
             + (X[b] @ w2[:D])[x,o] + (Y[b] @ w2[D:2D])[y,o] + w2[2D][o]

Sharding: tensor-parallel over o (the w1/w2 out_size axis): core c owns
o in [16c, 16c+16). Each core reads the full (transposed) inputs, its w1/w2
slice, and writes out[b, o_local, x, y]; the host reorders to [b,x,y,o].

Per-core schedule, per (o, batch-pair):
  stage1: M[j, x2] = sum_i W_o[i,j] XT[i, x2]        (PE, fp32r, N=512)
  fold:   M += w2b[j,o]      (DVE tensor_scalar_add on PSUM->SBUF copy)
  stage2: out[x, y] = sum_j M[j, x] YT[j, y]         (PE, fp32r, N=256)
  fold:   out += c1b[x, o]   (ACT Identity+bias on PSUM->SBUF copy)
where c1b = X[b] @ w2a + bias is computed once per (b, x-tile) upfront.
"""

import numpy as np

B, L, D, O = 8, 256, 512, 128
NCORES = 8
OS = O // NCORES     # 16 o-channels per core
IC = D // 128        # 4 contraction chunks of 128
NBP = B // 2         # 4 batch pairs (stage1 moving dim = 2*L = 512)

_CACHE = {}


def _build(mmdt_name: str, n_reps: int = 1, variant: str = "full"):
    """Build + compile the per-core Bass program (same program on all cores).

    n_reps > 1 repeats the main loop inside the NEFF (timing amplification
    for benchmarking only; results are identical since it rewrites the same
    outputs).
    """
    import concourse.tile as tile
    from concourse import bacc, mybir

    key = (mmdt_name, n_reps, variant)
    if key in _CACHE:
        return _CACHE[key]

    F32 = mybir.dt.float32
    MMDT = getattr(mybir.dt, mmdt_name)

    nc = bacc.Bacc("TRN2", target_bir_lowering=False, debug=False,
                   num_devices=NCORES)

    x1t_d = nc.dram_tensor("x1t", [B, D, L], MMDT, kind="ExternalInput")
    x2t_d = nc.dram_tensor("x2t", [B, D, L], MMDT, kind="ExternalInput")
    w1s_d = nc.dram_tensor("w1s", [OS, IC, 128, D], MMDT, kind="ExternalInput")
    w2a_d = nc.dram_tensor("w2a", [128, IC, OS], MMDT, kind="ExternalInput")
    w2b_d = nc.dram_tensor("w2b", [128, IC, OS], F32, kind="ExternalInput")
    bias_d = nc.dram_tensor("bias", [1, OS], MMDT, kind="ExternalInput")
    ones_d = nc.dram_tensor("ones", [1, 128], MMDT, kind="ExternalInput")
    out_d = nc.dram_tensor("out", [B, OS, L, L], F32, kind="ExternalOutput")


    if variant == "stub":
        # minimal program with identical I/O signature (absolute-timing baseline)
        with tile.TileContext(nc) as tc:
            with tc.tile_pool(name="sb", bufs=1) as sb:
                t = sb.tile([128, IC, OS], F32, tag="t")
                o_sb = sb.tile([128, L], F32, tag="o")
                nc.sync.dma_start(out=t, in_=w2b_d.ap())
                nc.vector.memset(o_sb, 0.0)
                nc.vector.tensor_scalar_add(o_sb[:, 0:OS * IC],
                                            o_sb[:, 0:OS * IC], t[:, 0, 0:1])
                nc.sync.dma_start(out=out_d.ap()[0, 0, 0:128, :], in_=o_sb)
        nc.compile()
        _CACHE[key] = nc
        return nc

    ps2_bufs = 6 if variant in ("v2", "o8", "v4", "v5", "v6") else 4
    with tile.TileContext(nc) as tc:
        with tc.tile_pool(name="small", bufs=1) as small, \
             tc.tile_pool(name="xy", bufs=1) as xy, \
             tc.tile_pool(name="wp", bufs=4) as wp, \
             tc.tile_pool(name="mp", bufs=3) as mp, \
             tc.tile_pool(name="op", bufs=8) as op, \
             tc.tile_pool(name="ps1", bufs=2, space="PSUM") as ps1:

            # --- small persistent tiles ---
            w2a_sb = small.tile([128, IC, OS], MMDT, tag="w2a")
            w2b_sb = small.tile([128, IC, OS], F32, tag="w2b")
            bias_sb = small.tile([1, OS], MMDT, tag="bias")
            ones_sb = small.tile([1, 128], MMDT, tag="ones")
            c1b_sb = small.tile([128, B * 2 * OS], F32, tag="c1b")
            nc.sync.dma_start(out=w2a_sb, in_=w2a_d.ap())
            nc.sync.dma_start(out=w2b_sb, in_=w2b_d.ap())
            nc.sync.dma_start(out=bias_sb, in_=bias_d.ap())
            nc.sync.dma_start(out=ones_sb, in_=ones_d.ap())

            # --- prefetch first W tiles so the o=0 stage1 isn't gated on the
            # (larger) input loads finishing first on the same DMA queue ---
            w_cache = {}
            # v5: W loads ride the Activation HWDGE ring so they never queue
            # behind the (in-order) sync ring's input loads + output stores
            w_eng = nc.scalar if variant == "v5" else nc.sync
            if variant in ("v2", "o8", "v3", "v4", "v5", "v6"):
                for o in range(2):
                    w_t = []
                    for ic in range(IC):
                        w = wp.tile([128, D], MMDT, tag=f"w{ic}")
                        w_eng.dma_start(out=w, in_=w1s_d.ap()[o, ic, :, :])
                        w_t.append(w)
                    w_cache[o] = w_t

            # --- transposed inputs, all batches resident: [i%128, ic, b_in, x] ---
            xts, yts = [], []
            for bp in range(NBP):
                xt = xy.tile([128, IC, 2, L], MMDT, tag=f"xt{bp}")
                yt = xy.tile([128, IC, 2, L], MMDT, tag=f"yt{bp}")
                for b_in in range(2):
                    b = 2 * bp + b_in
                    for ic in range(IC):
                        in_eng = nc.scalar if (variant == "v5" and ic % 2) \
                            else nc.sync
                        in_eng.dma_start(
                            out=xt[:, ic, b_in, :],
                            in_=x1t_d.ap()[b, ic * 128:(ic + 1) * 128, :])
                        in_eng.dma_start(
                            out=yt[:, ic, b_in, :],
                            in_=x2t_d.ap()[b, ic * 128:(ic + 1) * 128, :])
                xts.append(xt)
                yts.append(yt)

            # --- c1b[x, (b,xt,o)] = X[b] @ w2a + bias ---
            psc_pool = [None]

            def emit_c1b(b):
                psc = psc_pool[0]
                bp, b_in = divmod(b, 2)
                for xt_i in range(2):
                    # v4 borrows stage2's psum slots (same tag) so no extra
                    # PSUM banks are reserved for this startup-only work
                    pc = psc.tile([128, OS], F32,
                                  tag="p2" if variant == "v4" else "pc")
                    for ic in range(IC):
                        nc.tensor.matmul(
                            pc,
                            xts[bp][:, ic, b_in, xt_i * 128:(xt_i + 1) * 128],
                            w2a_sb[:, ic, :],
                            start=(ic == 0), stop=False)
                    nc.tensor.matmul(
                        pc, ones_sb[0:1, :], bias_sb[0:1, :],
                        start=False, stop=True)
                    nc.vector.tensor_copy(
                        c1b_sb[:, (b * 2 + xt_i) * OS:(b * 2 + xt_i + 1) * OS],
                        pc)

            if variant not in ("v3", "v4"):
                with tc.tile_pool(name="psc", bufs=2, space="PSUM") as psc:
                    psc_pool[0] = psc
                    for b in range(B):
                        emit_c1b(b)

            if variant == "v3":
                # c1b psum shares the pool budget with ps2 (2 + 4 + 2 = 8 banks)
                psc_ctx = tc.tile_pool(name="psc", bufs=2, space="PSUM")
                psc_pool[0] = psc_ctx.__enter__()
                ps2_bufs = 4
            ps2_ctx = tc.tile_pool(name="ps2", bufs=ps2_bufs, space="PSUM")
            ps2 = ps2_ctx.__enter__()
            if variant == "v4":
                psc_pool[0] = ps2

            # --- main loop: software-pipelined over (o, bp) ---
            def stage2(o, bp, m2):
                for b_in in range(2):
                    b = 2 * bp + b_in
                    for xt_i in range(2):
                        p2 = ps2.tile([128, L], F32, tag="p2")
                        for jc in range(IC):
                            nc.tensor.matmul(
                                p2,
                                m2[:, jc, b_in, xt_i * 128:(xt_i + 1) * 128],
                                yts[bp][:, jc, b_in, :],
                                start=(jc == 0), stop=(jc == IC - 1))
                        o_sb = op.tile([128, L], F32, tag="osb")
                        c1col = c1b_sb[:, (b * 2 + xt_i) * OS + o:
                                       (b * 2 + xt_i) * OS + o + 1]
                        if variant == "dvecopy":
                            nc.vector.tensor_scalar_add(o_sb, p2, c1col)
                        elif variant == "v6" and xt_i == 1:
                            # balance stage2 psum drains across DVE and ACT so
                            # neither engine gates ps2 slot reuse
                            nc.vector.tensor_scalar_add(o_sb, p2, c1col)
                        else:
                            nc.scalar.add(o_sb, p2, c1col)
                        if variant != "nodma" or (o == OS - 1 and bp == NBP - 1):
                            nc.sync.dma_start(
                                out=out_d.ap()[b, o, xt_i * 128:(xt_i + 1) * 128, :],
                                in_=o_sb)

            def stage1(o, bp, w_t):
                m2 = mp.tile([128, IC, 2, L], MMDT, tag="m2")
                for jt in range(IC):
                    p1 = ps1.tile([128, 2 * L], F32, tag="p1")
                    for ic in range(IC):
                        nc.tensor.matmul(
                            p1,
                            w_t[ic][:, jt * 128:(jt + 1) * 128],
                            xts[bp][:, ic, :, :],
                            start=(ic == 0), stop=(ic == IC - 1))
                    nc.vector.tensor_scalar_add(
                        m2[:, jt, :, :], p1, w2b_sb[:, jt, o:o + 1])
                return m2

            os_eff = OS // 2 if variant == "o8" else OS

            def emit_main():
                # software-pipelined: stage2 for (o,bp) runs one step behind
                # stage1 so the PE never waits on the DVE M-copies.
                prev = None
                for o in range(os_eff):
                    if o in w_cache:
                        w_t = w_cache.pop(o)
                    else:
                        w_t = []
                        for ic in range(IC):
                            w = wp.tile([128, D], MMDT, tag=f"w{ic}")
                            w_eng.dma_start(out=w, in_=w1s_d.ap()[o, ic, :, :])
                            w_t.append(w)
                    for bp in range(NBP):
                        m2 = stage1(o, bp, w_t)
                        if variant in ("v3", "v4") and o == 0:
                            emit_c1b(2 * bp)
                            emit_c1b(2 * bp + 1)
                        if prev is not None:
                            stage2(*prev)
                        prev = (o, bp, m2)
                stage2(*prev)

            if n_reps == 1:
                emit_main()
            else:
                with tc.For_i(0, n_reps, 1):
                    emit_main()
            ps2_ctx.__exit__(None, None, None)
            if variant == "v3":
                psc_ctx.__exit__(None, None, None)

    nc.compile()
    _CACHE[key] = nc
    return nc


def _build2(n_reps: int = 1, variant: str = "b16", timing: bool = False,
            os_eff: int = OS):
    """bf16 rewrite: same two-stage schedule as v2, with
      - all matmul operands bf16 (host converts; PE still 1 cyc/row but
        operand/weight-load SBUF bandwidth halves — the fp32r stage2
        N=256 case is right at the fp32r ldweights-hiding margin),
      - bf16 output stores (host upconverts; halves the dominant DMA
        stream), fp32 accumulation throughout (PSUM),
      - coalesced input/weight DMAs (one descriptor-chain per tile),
      - w1 tiles ride the ACT HWDGE ring, prefetch depth 3,
      - c1b warmup interleaved into the o=0 stage1 loop (v4-style).

    timing=True builds a NEFF with Internal (scratch) DRAM for the big
    tensors and a tiny external output, so paired wall-clock timing
    doesn't ship ~450MB through the axon tunnel per call. The main-loop
    instruction stream is identical to timing=False.
    """
    import concourse.tile as tile
    from concourse import bacc, mybir

    key = ("b16", n_reps, variant, timing, os_eff)
    if key in _CACHE:
        return _CACHE[key]

    F32 = mybir.dt.float32
    BF16 = mybir.dt.bfloat16

    nc = bacc.Bacc("TRN2", target_bir_lowering=False, debug=False,
                   num_devices=NCORES)

    big = dict(kind="Internal") if timing else dict(kind="ExternalInput")
    big_out = dict(kind="Internal") if timing else dict(kind="ExternalOutput")

    # host-marshaled layouts: partition dim (128) first, fully contiguous
    # per-partition payload per DMA
    x1t_d = nc.dram_tensor("x1t", [128, NBP, IC, 2, L], BF16, **big)
    x2t_d = nc.dram_tensor("x2t", [128, NBP, IC, 2, L], BF16, **big)
    w1s_d = nc.dram_tensor("w1s", [128, OS, IC, D], BF16, **big)
    w2a_d = nc.dram_tensor("w2a", [128, IC, OS], BF16, kind="ExternalInput")
    w2b_d = nc.dram_tensor("w2b", [128, IC, OS], F32, kind="ExternalInput")
    bias_d = nc.dram_tensor("bias", [1, OS], BF16, kind="ExternalInput")
    ones_d = nc.dram_tensor("ones", [1, 128], BF16, kind="ExternalInput")
    out_d = nc.dram_tensor("out", [B, OS, L, L], BF16, **big_out)
    if timing:
        tout_d = nc.dram_tensor("tout", [1, 128], BF16, kind="ExternalOutput")

    with tile.TileContext(nc) as tc:
        with tc.tile_pool(name="small", bufs=1) as small, \
             tc.tile_pool(name="xy", bufs=1) as xy, \
             tc.tile_pool(name="wp", bufs=4) as wp, \
             tc.tile_pool(name="mp", bufs=6) as mp, \
             tc.tile_pool(name="op", bufs=12) as op, \
             tc.tile_pool(name="ps1", bufs=3, space="PSUM") as ps1, \
             tc.tile_pool(name="ps2", bufs=5, space="PSUM") as ps2:

            # --- PE p-state warmup: zero matmuls, gated only on one
            # memset; their tiles are never read, so the body may recycle
            # the ps1 slots freely ---
            dum = small.tile([128, 128], BF16, tag="dum", name="dum")
            nc.vector.memset(dum, 0.0)
            for _ in range(24):
                pd = ps1.tile([128, 2 * L], F32, tag="p1", name="p1")
                nc.tensor.matmul(pd[:, 0:128], dum, dum,
                                 start=True, stop=True)

            # --- bp0 inputs lead the sync ring: stage1(o=0,bp=0) is gated
            # only on xt0 + w(o=0), everything else loads behind them ---
            xts, yts = [], []
            for bp in range(NBP):
                xts.append(xy.tile([128, IC, 2, L], BF16, tag=f"xt{bp}",
                                   name=f"xt{bp}"))
                yts.append(xy.tile([128, IC, 2, L], BF16, tag=f"yt{bp}",
                                   name=f"yt{bp}"))
            for ic in range(IC):
                nc.sync.dma_start(out=xts[0][:, ic], in_=x1t_d.ap()[:, 0, ic])

            # --- w1 tiles ride the ACT ring (never queue behind inputs);
            # the first tile is split across the ACT+Pool rings so its
            # transfer overlaps the xt0 load ---
            def load_w(o, split=False):
                w = wp.tile([128, IC, D], BF16, tag="w", name="w")
                if split:
                    nc.scalar.dma_start(out=w[:, 0:2, :],
                                        in_=w1s_d.ap()[:, o, 0:2, :])
                    nc.gpsimd.dma_start(out=w[:, 2:4, :],
                                        in_=w1s_d.ap()[:, o, 2:4, :])
                else:
                    nc.scalar.dma_start(out=w, in_=w1s_d.ap()[:, o])
                return w
            w_fifo = [load_w(o, split=(o == 0))
                      for o in range(min(3, os_eff))]

            # --- small persistent tiles ride the Pool ring (own DGE) ---
            w2a_sb = small.tile([128, IC, OS], BF16, tag="w2a")
            w2b_sb = small.tile([128, IC, OS], F32, tag="w2b")
            bias_sb = small.tile([1, OS], BF16, tag="bias")
            ones_sb = small.tile([1, 128], BF16, tag="ones")
            c1b_sb = small.tile([128, B * 2 * OS], F32, tag="c1b")
            nc.gpsimd.dma_start(out=w2a_sb, in_=w2a_d.ap())
            nc.gpsimd.dma_start(out=w2b_sb, in_=w2b_d.ap())
            nc.gpsimd.dma_start(out=bias_sb, in_=bias_d.ap())
            nc.gpsimd.dma_start(out=ones_sb, in_=ones_d.ap())

            # --- remaining inputs; xt1 ahead of yt0 (stage1(0,bp1)
            # consumes it before stage2(0,bp0) needs yt0) ---
            nc.sync.dma_start(out=xts[1], in_=x1t_d.ap()[:, 1])
            nc.sync.dma_start(out=yts[0], in_=x2t_d.ap()[:, 0])
            nc.sync.dma_start(out=yts[1], in_=x2t_d.ap()[:, 1])
            for bp in range(2, NBP):
                nc.sync.dma_start(out=xts[bp], in_=x1t_d.ap()[:, bp])
                nc.sync.dma_start(out=yts[bp], in_=x2t_d.ap()[:, bp])

            def emit_c1b(b):
                bp, b_in = divmod(b, 2)
                for xt_i in range(2):
                    pc = ps2.tile([128, L], F32, tag="p2")
                    for ic in range(IC):
                        nc.tensor.matmul(
                            pc[:, 0:OS],
                            xts[bp][:, ic, b_in, xt_i * 128:(xt_i + 1) * 128],
                            w2a_sb[:, ic, :],
                            start=(ic == 0), stop=False)
                    nc.tensor.matmul(
                        pc[:, 0:OS], ones_sb[0:1, :], bias_sb[0:1, :],
                        start=False, stop=True)
                    nc.vector.tensor_copy(
                        c1b_sb[:, (b * 2 + xt_i) * OS:(b * 2 + xt_i + 1) * OS],
                        pc[:, 0:OS])

            def stage1(o, bp, w_t):
                m2 = mp.tile([128, IC, 2, L], BF16, tag="m2")
                for jt in range(IC):
                    p1 = ps1.tile([128, 2 * L], F32, tag="p1")
                    for ic in range(IC):
                        nc.tensor.matmul(
                            p1,
                            w_t[:, ic, jt * 128:(jt + 1) * 128],
                            xts[bp][:, ic, :, :],
                            start=(ic == 0), stop=(ic == IC - 1))
                    nc.vector.tensor_scalar_add(
                        m2[:, jt, :, :], p1, w2b_sb[:, jt, o:o + 1])
                return m2

            def stage2(o, bp, m2):
                for b_in in range(2):
                    b = 2 * bp + b_in
                    for xt_i in range(2):
                        p2 = ps2.tile([128, L], F32, tag="p2")
                        for jc in range(IC):
                            nc.tensor.matmul(
                                p2,
                                m2[:, jc, b_in, xt_i * 128:(xt_i + 1) * 128],
                                yts[bp][:, jc, b_in, :],
                                start=(jc == 0), stop=(jc == IC - 1))
                        o_sb = op.tile([128, L], BF16, tag="osb")
                        c1col = c1b_sb[:, (b * 2 + xt_i) * OS + o:
                                       (b * 2 + xt_i) * OS + o + 1]
                        nc.scalar.add(o_sb, p2, c1col)
                        nc.sync.dma_start(
                            out=out_d.ap()[b, o, xt_i * 128:(xt_i + 1) * 128, :],
                            in_=o_sb)
                return o_sb

            last_osb = [None]

            def emit_main():
                prev = None
                for o in range(os_eff):
                    w_t = w_fifo[o % len(w_fifo)]
                    if o + 3 < os_eff:
                        w_fifo[(o + 3) % len(w_fifo)] = load_w(o + 3)
                    for bp in range(NBP):
                        m2 = stage1(o, bp, w_t)
                        if o == 0:
                            emit_c1b(2 * bp)
                            emit_c1b(2 * bp + 1)
                        if prev is not None:
                            stage2(*prev)
                        prev = (o, bp, m2)
                last_osb[0] = stage2(*prev)

            if n_reps == 1:
                emit_main()
            else:
                with tc.For_i(0, n_reps, 1):
                    emit_main()

            if timing:
                nc.sync.dma_start(out=tout_d.ap(),
                                  in_=last_osb[0][0:1, :][:, 0:128])

    nc.compile()
    _CACHE[key] = nc
    return nc


def _build3(n_reps: int = 1, timing: bool = False, os_eff: int = OS,
            warm: int = 24):
    """b16s: bf16 two-stage schedule with a swapped stage2.

    stage1 (unchanged): p1[j, (b_in,x)] = sum_i W_o[i,j] X[i, (b_in,x)],
      DVE drains +w2b[j,o] into m2[j, jc, b_in, oi, x] (bf16), two o's
      (an "o-pair") share one m2 tile.
    stage2 (swapped): stationary = Y chunk [j, y128], moving = m2 slice
      [j, (oi,x)=512] -> p2[y128, (oi,x)] accumulated over 4 j-chunks.
      vs the b16 layout this halves stage2 matmul/ldweights/drain/store
      counts and doubles the moving dim (better weight-load hiding).
    The x-affine term (X@w2a + bias) no longer folds as a per-partition
    scalar (partitions are now y), so the HOST adds it during unshard;
    the y-affine (Y@w2b) still rides stage1's +w2b fold.
    A short burst of dummy matmuls at t=0 ramps the PE p-state out of
    its cold clock while the first DMAs land.
    """
    import concourse.tile as tile
    from concourse import bacc, mybir

    key = ("b16s", n_reps, timing, os_eff, warm)
    if key in _CACHE:
        return _CACHE[key]

    F32 = mybir.dt.float32
    BF16 = mybir.dt.bfloat16
    OP2 = os_eff // 2

    nc = bacc.Bacc("TRN2", target_bir_lowering=False, debug=False,
                   num_devices=NCORES)

    big = dict(kind="Internal") if timing else dict(kind="ExternalInput")
    big_out = dict(kind="Internal") if timing else dict(kind="ExternalOutput")

    x1t_d = nc.dram_tensor("x1t", [128, NBP, IC, 2, L], BF16, **big)
    x2t_d = nc.dram_tensor("x2t", [128, NBP, IC, 2, L], BF16, **big)
    w1s_d = nc.dram_tensor("w1s", [128, OS, IC, D], BF16, **big)
    w2b_d = nc.dram_tensor("w2b", [128, IC, OS], F32, kind="ExternalInput")
    out_d = nc.dram_tensor("out", [B, OS // 2, 2, 128, 2 * L], BF16,
                           **big_out)
    if timing:
        tout_d = nc.dram_tensor("tout", [1, 128], BF16, kind="ExternalOutput")

    with tile.TileContext(nc) as tc:
        with tc.tile_pool(name="small", bufs=1) as small, \
             tc.tile_pool(name="xy", bufs=1) as xy, \
             tc.tile_pool(name="wp", bufs=4) as wp, \
             tc.tile_pool(name="mp", bufs=3) as mp, \
             tc.tile_pool(name="op", bufs=8) as op, \
             tc.tile_pool(name="ps1", bufs=3, space="PSUM") as ps1, \
             tc.tile_pool(name="ps2", bufs=4, space="PSUM") as ps2, \
             tc.tile_pool(name="psw", bufs=1, space="PSUM") as psw:

            # --- PE p-state warmup: zero matmuls, gated only on one memset ---
            dum = small.tile([128, 128], BF16, tag="dum", name="dum")
            nc.vector.memset(dum, 0.0)
            for _ in range(warm):
                pd = psw.tile([128, 2 * L], F32, tag="pw", name="pw")
                nc.tensor.matmul(pd[:, 0:128], dum, dum,
                                 start=True, stop=True)

            # --- bp0 inputs lead the sync ring, per-ic for fine gating ---
            xts, yts = [], []
            for bp in range(NBP):
                xts.append(xy.tile([128, IC, 2, L], BF16, tag=f"xt{bp}",
                                   name=f"xt{bp}"))
                yts.append(xy.tile([128, IC, 2, L], BF16, tag=f"yt{bp}",
                                   name=f"yt{bp}"))
            for ic in range(IC):
                nc.sync.dma_start(out=xts[0][:, ic], in_=x1t_d.ap()[:, 0, ic])

            # --- w1 tiles on the ACT ring; first split ACT+Pool ---
            def load_w(o, split=False):
                w = wp.tile([128, IC, D], BF16, tag="w", name="w")
                if split:
                    nc.scalar.dma_start(out=w[:, 0:2, :],
                                        in_=w1s_d.ap()[:, o, 0:2, :])
                    nc.gpsimd.dma_start(out=w[:, 2:4, :],
                                        in_=w1s_d.ap()[:, o, 2:4, :])
                else:
                    nc.scalar.dma_start(out=w, in_=w1s_d.ap()[:, o])
                return w
            # n_reps==1: prefetch 3 of 4 slots before the main loop.
            # n_reps>1 (timing builds): NO pre-loop w tiles — tiles read
            # inside a For_i body but loaded outside are pinned for the
            # loop's lifetime, so in-body loads couldn't recycle their
            # slots; instead every load (16/rep, matching the real
            # kernel's DMA) is emitted inside the body.
            w_tiles = {}
            if n_reps == 1:
                for o in range(min(3, os_eff)):
                    w_tiles[o] = load_w(o, split=(o == 0))

            w2b_sb = small.tile([128, IC, OS], F32, tag="w2b")
            nc.gpsimd.dma_start(out=w2b_sb, in_=w2b_d.ap())

            # xt1 ahead of yt0: stage1(0,bp1) consumes it well before
            # stage2(0,bp0) needs yt0
            nc.sync.dma_start(out=xts[1], in_=x1t_d.ap()[:, 1])
            nc.sync.dma_start(out=yts[0], in_=x2t_d.ap()[:, 0])
            nc.sync.dma_start(out=yts[1], in_=x2t_d.ap()[:, 1])
            for bp in range(2, NBP):
                nc.sync.dma_start(out=xts[bp], in_=x1t_d.ap()[:, bp])
                nc.sync.dma_start(out=yts[bp], in_=x2t_d.ap()[:, bp])

            def stage1(o, bp, w_t, m2, oi):
                for jt in range(IC):
                    p1 = ps1.tile([128, 2 * L], F32, tag="p1", name="p1")
                    for ic in range(IC):
                        nc.tensor.matmul(
                            p1,
                            w_t[:, ic, jt * 128:(jt + 1) * 128],
                            xts[bp][:, ic, :, :],
                            start=(ic == 0), stop=(ic == IC - 1))
                    nc.vector.tensor_scalar_add(
                        m2[:, jt, :, oi, :], p1, w2b_sb[:, jt, o:o + 1])

            def stage2(opair, bp, m2):
                for b_in in range(2):
                    b = 2 * bp + b_in
                    for yc in range(2):
                        p2 = ps2.tile([128, 2 * L], F32, tag="p2", name="p2")
                        for jc in range(IC):
                            nc.tensor.matmul(
                                p2,
                                yts[bp][:, jc, b_in, yc * 128:(yc + 1) * 128],
                                m2[:, jc, b_in, :, :],
                                start=(jc == 0), stop=(jc == IC - 1))
                        o_sb = op.tile([128, 2 * L], BF16, tag="osb",
                                       name="osb")
                        nc.scalar.copy(o_sb, p2)
                        nc.sync.dma_start(out=out_d.ap()[b, opair, yc],
                                          in_=o_sb)
                return o_sb

            last_osb = [None]

            def emit_main(in_loop):
                prev = None
                if in_loop:
                    for o in range(min(4, os_eff)):
                        w_tiles[o] = load_w(o)
                for opair in range(OP2):
                    o0, o1 = 2 * opair, 2 * opair + 1
                    wa, wb = w_tiles[o0], w_tiles[o1]
                    for bp in range(NBP):
                        m2 = mp.tile([128, IC, 2, 2, L], BF16, tag="m2",
                                     name="m2")
                        stage1(o0, bp, wa, m2, 0)
                        stage1(o1, bp, wb, m2, 1)
                        if prev is not None:
                            stage2(*prev)
                        prev = (opair, bp, m2)
                    # issue loads only after every stage1 read of the slot
                    # they recycle (keeps each wp slot read-before-write
                    # within the For_i body, which the tile scheduler
                    # requires)
                    base = 2 * opair + (4 if in_loop else 3)
                    for oo in (base, base + 1):
                        if oo < os_eff and oo not in w_tiles:
                            w_tiles[oo] = load_w(oo)
                last_osb[0] = stage2(*prev)

            if n_reps == 1:
                emit_main(False)
            else:
                with tc.For_i(0, n_reps, 1):
                    emit_main(True)

            if timing:
                nc.sync.dma_start(out=tout_d.ap(),
                                  in_=last_osb[0][0:1, 0:128])

    nc.compile()
    _CACHE[key] = nc
    return nc


def make_in_maps3(input1, input2, w1, w2, timing: bool = False):
    """Host-side marshaling for _build3 (b16s)."""
    import ml_dtypes

    bf16 = ml_dtypes.bfloat16
    input1 = np.asarray(input1, dtype=np.float32)
    input2 = np.asarray(input2, dtype=np.float32)
    w1 = np.asarray(w1, dtype=np.float32)
    w2 = np.asarray(w2, dtype=np.float32)

    def xmarsh(x):
        v = x.reshape(NBP, 2, L, IC, 128)          # [bp, b_in, l, ic, p]
        return np.ascontiguousarray(
            v.transpose(4, 0, 3, 1, 2)).astype(bf16)  # [p, bp, ic, b_in, l]

    x1t = xmarsh(input1)
    x2t = xmarsh(input2)

    in_maps = []
    for c in range(NCORES):
        sl = slice(c * OS, (c + 1) * OS)
        w2b = np.ascontiguousarray(
            w2[D:2 * D, sl].reshape(IC, 128, OS).transpose(1, 0, 2))
        m = {"w2b": w2b}
        if not timing:
            w1s = np.ascontiguousarray(
                w1[:, sl, :].reshape(IC, 128, OS, D).transpose(1, 2, 0, 3)
            ).astype(bf16)
            m.update({"x1t": x1t, "x2t": x2t, "w1s": w1s})
        in_maps.append(m)
    return in_maps


def make_in_maps2(input1, input2, w1, w2, timing: bool = False):
    """Host-side marshaling for _build2 (bf16, partition-major layouts)."""
    import ml_dtypes

    bf16 = ml_dtypes.bfloat16
    input1 = np.asarray(input1, dtype=np.float32)
    input2 = np.asarray(input2, dtype=np.float32)
    w1 = np.asarray(w1, dtype=np.float32)
    w2 = np.asarray(w2, dtype=np.float32)

    # x1t[p, bp, ic, b_in, l] = input1[2bp+b_in, l, ic*128+p]
    def xmarsh(x):
        v = x.reshape(NBP, 2, L, IC, 128)          # [bp, b_in, l, ic, p]
        return np.ascontiguousarray(
            v.transpose(4, 0, 3, 1, 2)).astype(bf16)  # [p, bp, ic, b_in, l]

    x1t = xmarsh(input1)
    x2t = xmarsh(input2)
    ones = np.ones((1, 128), dtype=bf16)

    in_maps = []
    for c in range(NCORES):
        sl = slice(c * OS, (c + 1) * OS)
        w2a = np.ascontiguousarray(
            w2[:D, sl].reshape(IC, 128, OS).transpose(1, 0, 2)).astype(bf16)
        w2b = np.ascontiguousarray(
            w2[D:2 * D, sl].reshape(IC, 128, OS).transpose(1, 0, 2))
        bias = np.ascontiguousarray(w2[2 * D:2 * D + 1, sl]).astype(bf16)
        m = {"w2a": w2a, "w2b": w2b, "bias": bias, "ones": ones}
        if not timing:
            # w1s[p, o, ic, j] = w1[ic*128+p, o_global, j]
            w1s = np.ascontiguousarray(
                w1[:, sl, :].reshape(IC, 128, OS, D).transpose(1, 2, 0, 3)
            ).astype(bf16)
            m.update({"x1t": x1t, "x2t": x2t, "w1s": w1s})
        in_maps.append(m)
    return in_maps


def make_in_maps(input1, input2, w1, w2):
    """Host-side data marshaling (sharding + layout)."""
    input1 = np.asarray(input1, dtype=np.float32)
    input2 = np.asarray(input2, dtype=np.float32)
    w1 = np.asarray(w1, dtype=np.float32)
    w2 = np.asarray(w2, dtype=np.float32)

    x1t = np.ascontiguousarray(input1.transpose(0, 2, 1))      # [B, D, L]
    x2t = np.ascontiguousarray(input2.transpose(0, 2, 1))      # [B, D, L]
    ones = np.ones((1, 128), dtype=np.float32)

    in_maps = []
    for c in range(NCORES):
        sl = slice(c * OS, (c + 1) * OS)
        w1s = np.ascontiguousarray(
            w1[:, sl, :].transpose(1, 0, 2)).reshape(OS, IC, 128, D)
        # SBUF layout is [i_in_chunk(128 partitions), chunk, o]
        w2a = np.ascontiguousarray(
            w2[:D, sl].reshape(IC, 128, OS).transpose(1, 0, 2))
        w2b = np.ascontiguousarray(
            w2[D:2 * D, sl].reshape(IC, 128, OS).transpose(1, 0, 2))
        bias = np.ascontiguousarray(w2[2 * D:2 * D + 1, sl])
        in_maps.append({"x1t": x1t, "x2t": x2t, "w1s": w1s,
                        "w2a": w2a, "w2b": w2b, "bias": bias, "ones": ones})
    return in_maps


def kernel(input1, input2, w1, w2):
    from concourse.bass_utils import run_bass_kernel_spmd

    in_maps = make_in_maps2(input1, input2, w1, w2)
    nc = _build2(1, "b16")
    res = run_bass_kernel_spmd(nc, in_maps, core_ids=list(range(NCORES)))

    out = np.empty((B, L, L, O), dtype=np.float32)
    for c in range(NCORES):
        # per-core result is [B, OS, L, L] bf16 -> [B, L, L, OS] f32
        out[:, :, :, c * OS:(c + 1) * OS] = \
            np.asarray(res.results[c]["out"]).astype(np.float32) \
            .transpose(0, 2, 3, 1)
    return out



# revision 26
# speedup vs baseline: 1.1664x; 1.0152x over previous
"""Biaffine kernel for Trainium2 (8 NeuronCores, Bass/Tile).

out[b,x,y,o] = sum_ij X[b,x,i] w1[i,o,j] Y[b,y,j]
             + (X[b] @ w2[:D])[x,o] + (Y[b] @ w2[D:2D])[y,o] + w2[2D][o]

Sharding: tensor-parallel over o (the w1/w2 out_size axis): core c owns
o in [16c, 16c+16). Each core reads the full (transposed) inputs, its
w1/w2 slice, and writes out[b, o_local, x, y] in bf16; the host reorders
to [b,x,y,o] and upconverts to f32.

All matmul operands are bf16 (host-converted; rel err ~4e-3 vs the 2e-2
gate), accumulation is fp32 in PSUM. bf16 halves PE operand/weight-load
SBUF bandwidth (fp32r stage2 at N=256 sat right at the ldweights-hiding
margin) and halves the dominant DMA streams (w1 in, out stores).

Per-core schedule, per (o, batch-pair):
  stage1: M[j, x2] = sum_i W_o[i,j] XT[i, x2]        (PE, bf16, N=512)
  fold:   M += w2b[j,o]      (DVE tensor_scalar_add on PSUM->SBUF drain)
  stage2: out[x, y] = sum_j M[j, x] YT[j, y]         (PE, bf16, N=256)
  fold:   out += c1b[x, o]   (ACT add on PSUM->SBUF drain)
where c1b = X[b] @ w2a + bias is computed during the o=0 stage1 loop.
Per-core: 786,432 PE cycles = 327.7us at 2.4GHz; measured ~339us/rep
(steady state) + ~15us startup/tail.
"""

import numpy as np

B, L, D, O = 8, 256, 512, 128
NCORES = 8
OS = O // NCORES     # 16 o-channels per core
IC = D // 128        # 4 contraction chunks of 128
NBP = B // 2         # 4 batch pairs (stage1 moving dim = 2*L = 512)

_CACHE = {}


def _build2(n_reps: int = 1, variant: str = "b16", timing: bool = False,
            os_eff: int = OS):
    """bf16 rewrite: same two-stage schedule as v2, with
      - all matmul operands bf16 (host converts; PE still 1 cyc/row but
        operand/weight-load SBUF bandwidth halves — the fp32r stage2
        N=256 case is right at the fp32r ldweights-hiding margin),
      - bf16 output stores (host upconverts; halves the dominant DMA
        stream), fp32 accumulation throughout (PSUM),
      - coalesced input/weight DMAs (one descriptor-chain per tile),
      - w1 tiles ride the ACT HWDGE ring, prefetch depth 3,
      - c1b warmup interleaved into the o=0 stage1 loop (v4-style).

    timing=True builds a NEFF with Internal (scratch) DRAM for the big
    tensors and a tiny external output, so paired wall-clock timing
    doesn't ship ~450MB through the axon tunnel per call. The main-loop
    instruction stream is identical to timing=False.
    """
    import concourse.tile as tile
    from concourse import bacc, mybir

    key = ("b16", n_reps, variant, timing, os_eff)
    if key in _CACHE:
        return _CACHE[key]

    F32 = mybir.dt.float32
    BF16 = mybir.dt.bfloat16

    nc = bacc.Bacc("TRN2", target_bir_lowering=False, debug=False,
                   num_devices=NCORES)

    big = dict(kind="Internal") if timing else dict(kind="ExternalInput")
    big_out = dict(kind="Internal") if timing else dict(kind="ExternalOutput")

    # host-marshaled layouts: partition dim (128) first, fully contiguous
    # per-partition payload per DMA
    x1t_d = nc.dram_tensor("x1t", [128, NBP, IC, 2, L], BF16, **big)
    x2t_d = nc.dram_tensor("x2t", [128, NBP, IC, 2, L], BF16, **big)
    w1s_d = nc.dram_tensor("w1s", [128, OS, IC, D], BF16, **big)
    w2a_d = nc.dram_tensor("w2a", [128, IC, OS], BF16, kind="ExternalInput")
    w2b_d = nc.dram_tensor("w2b", [128, IC, OS], F32, kind="ExternalInput")
    bias_d = nc.dram_tensor("bias", [1, OS], BF16, kind="ExternalInput")
    ones_d = nc.dram_tensor("ones", [1, 128], BF16, kind="ExternalInput")
    out_d = nc.dram_tensor("out", [B, OS, L, L], BF16, **big_out)
    if timing:
        tout_d = nc.dram_tensor("tout", [1, 128], BF16, kind="ExternalOutput")

    with tile.TileContext(nc) as tc:
        with tc.tile_pool(name="small", bufs=1) as small, \
             tc.tile_pool(name="xy", bufs=1) as xy, \
             tc.tile_pool(name="wp", bufs=4) as wp, \
             tc.tile_pool(name="mp", bufs=6) as mp, \
             tc.tile_pool(name="op", bufs=12) as op, \
             tc.tile_pool(name="ps1", bufs=3, space="PSUM") as ps1, \
             tc.tile_pool(name="ps2", bufs=5, space="PSUM") as ps2:

            # --- PE p-state warmup: zero matmuls, gated only on one
            # memset; their tiles are never read, so the body may recycle
            # the ps1 slots freely ---
            dum = small.tile([128, 128], BF16, tag="dum", name="dum")
            nc.vector.memset(dum, 0.0)
            for _ in range(24):
                pd = ps1.tile([128, 2 * L], F32, tag="p1", name="p1")
                nc.tensor.matmul(pd[:, 0:128], dum, dum,
                                 start=True, stop=True)

            # --- bp0 inputs lead the sync ring: stage1(o=0,bp=0) is gated
            # only on xt0 + w(o=0), everything else loads behind them ---
            xts, yts = [], []
            for bp in range(NBP):
                xts.append(xy.tile([128, IC, 2, L], BF16, tag=f"xt{bp}",
                                   name=f"xt{bp}"))
                yts.append(xy.tile([128, IC, 2, L], BF16, tag=f"yt{bp}",
                                   name=f"yt{bp}"))
            for ic in range(IC):
                nc.sync.dma_start(out=xts[0][:, ic], in_=x1t_d.ap()[:, 0, ic])

            # --- w1 tiles ride the ACT ring (never queue behind inputs);
            # the first tile is split across the ACT+Pool rings so its
            # transfer overlaps the xt0 load ---
            def load_w(o, split=False):
                w = wp.tile([128, IC, D], BF16, tag="w", name="w")
                if split:
                    nc.scalar.dma_start(out=w[:, 0:2, :],
                                        in_=w1s_d.ap()[:, o, 0:2, :])
                    nc.gpsimd.dma_start(out=w[:, 2:4, :],
                                        in_=w1s_d.ap()[:, o, 2:4, :])
                else:
                    nc.scalar.dma_start(out=w, in_=w1s_d.ap()[:, o])
                return w
            w_fifo = [load_w(o, split=(o == 0))
                      for o in range(min(3, os_eff))]

            # --- small persistent tiles ride the Pool ring (own DGE) ---
            w2a_sb = small.tile([128, IC, OS], BF16, tag="w2a")
            w2b_sb = small.tile([128, IC, OS], F32, tag="w2b")
            bias_sb = small.tile([1, OS], BF16, tag="bias")
            ones_sb = small.tile([1, 128], BF16, tag="ones")
            c1b_sb = small.tile([128, B * 2 * OS], F32, tag="c1b")
            nc.gpsimd.dma_start(out=w2a_sb, in_=w2a_d.ap())
            nc.gpsimd.dma_start(out=w2b_sb, in_=w2b_d.ap())
            nc.gpsimd.dma_start(out=bias_sb, in_=bias_d.ap())
            nc.gpsimd.dma_start(out=ones_sb, in_=ones_d.ap())

            # --- remaining inputs; xt1 ahead of yt0 (stage1(0,bp1)
            # consumes it before stage2(0,bp0) needs yt0) ---
            nc.sync.dma_start(out=xts[1], in_=x1t_d.ap()[:, 1])
            nc.sync.dma_start(out=yts[0], in_=x2t_d.ap()[:, 0])
            nc.sync.dma_start(out=yts[1], in_=x2t_d.ap()[:, 1])
            for bp in range(2, NBP):
                nc.sync.dma_start(out=xts[bp], in_=x1t_d.ap()[:, bp])
                nc.sync.dma_start(out=yts[bp], in_=x2t_d.ap()[:, bp])

            def emit_c1b(b):
                bp, b_in = divmod(b, 2)
                for xt_i in range(2):
                    pc = ps2.tile([128, L], F32, tag="p2")
                    for ic in range(IC):
                        nc.tensor.matmul(
                            pc[:, 0:OS],
                            xts[bp][:, ic, b_in, xt_i * 128:(xt_i + 1) * 128],
                            w2a_sb[:, ic, :],
                            start=(ic == 0), stop=False)
                    nc.tensor.matmul(
                        pc[:, 0:OS], ones_sb[0:1, :], bias_sb[0:1, :],
                        start=False, stop=True)
                    nc.vector.tensor_copy(
                        c1b_sb[:, (b * 2 + xt_i) * OS:(b * 2 + xt_i + 1) * OS],
                        pc[:, 0:OS])

            def stage1(o, bp, w_t):
                m2 = mp.tile([128, IC, 2, L], BF16, tag="m2")
                for jt in range(IC):
                    p1 = ps1.tile([128, 2 * L], F32, tag="p1")
                    for ic in range(IC):
                        nc.tensor.matmul(
                            p1,
                            w_t[:, ic, jt * 128:(jt + 1) * 128],
                            xts[bp][:, ic, :, :],
                            start=(ic == 0), stop=(ic == IC - 1))
                    nc.vector.tensor_scalar_add(
                        m2[:, jt, :, :], p1, w2b_sb[:, jt, o:o + 1])
                return m2

            def stage2(o, bp, m2):
                for b_in in range(2):
                    b = 2 * bp + b_in
                    for xt_i in range(2):
                        p2 = ps2.tile([128, L], F32, tag="p2")
                        for jc in range(IC):
                            nc.tensor.matmul(
                                p2,
                                m2[:, jc, b_in, xt_i * 128:(xt_i + 1) * 128],
                                yts[bp][:, jc, b_in, :],
                                start=(jc == 0), stop=(jc == IC - 1))
                        o_sb = op.tile([128, L], BF16, tag="osb")
                        c1col = c1b_sb[:, (b * 2 + xt_i) * OS + o:
                                       (b * 2 + xt_i) * OS + o + 1]
                        nc.scalar.add(o_sb, p2, c1col)
                        nc.sync.dma_start(
                            out=out_d.ap()[b, o, xt_i * 128:(xt_i + 1) * 128, :],
                            in_=o_sb)
                return o_sb

            last_osb = [None]

            def emit_main():
                prev = None
                for o in range(os_eff):
                    w_t = w_fifo[o % len(w_fifo)]
                    if o + 3 < os_eff:
                        w_fifo[(o + 3) % len(w_fifo)] = load_w(o + 3)
                    for bp in range(NBP):
                        m2 = stage1(o, bp, w_t)
                        if o == 0:
                            emit_c1b(2 * bp)
                            emit_c1b(2 * bp + 1)
                        if prev is not None:
                            stage2(*prev)
                        prev = (o, bp, m2)
                last_osb[0] = stage2(*prev)

            if n_reps == 1:
                emit_main()
            else:
                with tc.For_i(0, n_reps, 1):
                    emit_main()

            if timing:
                nc.sync.dma_start(out=tout_d.ap(),
                                  in_=last_osb[0][0:1, :][:, 0:128])

    nc.compile()
    _CACHE[key] = nc
    return nc


def make_in_maps2(input1, input2, w1, w2, timing: bool = False):
    """Host-side marshaling for _build2 (bf16, partition-major layouts)."""
    import ml_dtypes

    bf16 = ml_dtypes.bfloat16
    input1 = np.asarray(input1, dtype=np.float32)
    input2 = np.asarray(input2, dtype=np.float32)
    w1 = np.asarray(w1, dtype=np.float32)
    w2 = np.asarray(w2, dtype=np.float32)

    # x1t[p, bp, ic, b_in, l] = input1[2bp+b_in, l, ic*128+p]
    def xmarsh(x):
        v = x.reshape(NBP, 2, L, IC, 128)          # [bp, b_in, l, ic, p]
        return np.ascontiguousarray(
            v.transpose(4, 0, 3, 1, 2)).astype(bf16)  # [p, bp, ic, b_in, l]

    x1t = xmarsh(input1)
    x2t = xmarsh(input2)
    ones = np.ones((1, 128), dtype=bf16)

    in_maps = []
    for c in range(NCORES):
        sl = slice(c * OS, (c + 1) * OS)
        w2a = np.ascontiguousarray(
            w2[:D, sl].reshape(IC, 128, OS).transpose(1, 0, 2)).astype(bf16)
        w2b = np.ascontiguousarray(
            w2[D:2 * D, sl].reshape(IC, 128, OS).transpose(1, 0, 2))
        bias = np.ascontiguousarray(w2[2 * D:2 * D + 1, sl]).astype(bf16)
        m = {"w2a": w2a, "w2b": w2b, "bias": bias, "ones": ones}
        if not timing:
            # w1s[p, o, ic, j] = w1[ic*128+p, o_global, j]
            w1s = np.ascontiguousarray(
                w1[:, sl, :].reshape(IC, 128, OS, D).transpose(1, 2, 0, 3)
            ).astype(bf16)
            m.update({"x1t": x1t, "x2t": x2t, "w1s": w1s})
        in_maps.append(m)
    return in_maps


def kernel(input1, input2, w1, w2):
    from concourse.bass_utils import run_bass_kernel_spmd

    in_maps = make_in_maps2(input1, input2, w1, w2)
    nc = _build2(1, "b16")
    res = run_bass_kernel_spmd(nc, in_maps, core_ids=list(range(NCORES)))

    out = np.empty((B, L, L, O), dtype=np.float32)
    for c in range(NCORES):
        # per-core result is [B, OS, L, L] bf16 -> [B, L, L, OS] f32
        out[:, :, :, c * OS:(c + 1) * OS] = \
            np.asarray(res.results[c]["out"]).astype(np.float32) \
            .transpose(0, 2, 3, 1)
    return out

